# revision 25
# baseline (speedup 1.0000x reference)
"""Trainium2 Bass kernel for BatchIrregularDownsample2d (D=2).

Contract: kernel(**inputs) takes the FULL inputs
    input:        [B, C, N]  float32
    pooling_mask: [B, 1, H, W] int32
and returns the FULL output [B, C, M] float32, where M is the max
per-batch compacted length (identical across batches for quadtree masks
with equal level histograms, which is what this module produces).

Strategy (pure data-parallel over B, one batch per NeuronCore):
  The reference gather G[b] splits into
    - an identity prefix  out[:, :start]         = in[:, :start]
    - a gather            out[:, start:start+ng] = in[:, start + rel[j]]
  The gather indices further split into a "dt" part (dont-touch tokens,
  sorted, and - because quadtree blocks contribute aligned 2-token units
  to the compacted stream - consisting entirely of even-aligned pairs
  (2c, 2c+1)) followed by a "kl" part (pooled representatives, sorted
  singles).

  Per core: the prefix is a DRAM->DRAM DMA; the gather source region
  [C=256, nelems] is DMA'd into SBUF in 8 staged chunks which the DVE
  interleaves elementwise into srcI[128, nelems, 2] (both 128-partition
  C-chunks per token).  The GPSIMD ap_gather op costs ~140 cycles per
  4-index request regardless of d, so the dt part is gathered with d=4
  over an aligned 2-token cell view of srcI (cell index c = pair (2c,
  2c+1); half the requests of a d=2 token gather), and the kl part with
  d=2 over the token view.  The DVE de-interleaves each gathered range
  into [2, n] plane-major bounce buffers that a single DMA per range
  stores to both 128-row halves of the output.  Index arithmetic is
  host-side numpy (as in the original torch module, which syncs the mask
  to host anyway).
"""

import numpy as np

from concourse import bass, library_config, mybir
from concourse.bass_utils import run_bass_kernel_spmd

f32 = mybir.dt.float32
i16 = mybir.dt.int16

_NUM_CORES = 8
_N_SUB = 8          # gather-region load sub-chunks
_DT_Q = 4           # positional sub-gathers over the dt cell stream


# ---------------------------------------------------------------------------
# Host-side index computation (replicates reference._build_indices, D=2)
# ---------------------------------------------------------------------------

def _batch_indices(mask2d):
    """mask2d: [H, W] int32 quadtree mask. Returns (start, rel_idx int64[ng])
    with absolute gather index = start + rel_idx."""
    D = 2
    s = 2 ** (D - 1)
    start = 0
    for i in range(D - 1):
        start += int((mask2d == i).sum()) // (4 ** i)
    cs = (mask2d >= D - 1)[::s, ::s]
    dt = (mask2d < D)[::s, ::s]
    r, c = np.nonzero(cs)
    topleft = ((r % 2) + (c % 2)) == 0
    dt_at = dt[r, c]
    keep_lower = topleft & ~dt_at
    pos = np.arange(r.shape[0])
    rel = np.concatenate([pos[dt_at], pos[keep_lower]]).astype(np.int64)
    return start, rel, int(r.shape[0])


def _split_dt_kl(rel):
    """rel = [dt part (sorted), kl part (sorted)]; returns (dt, kl)."""
    dec = np.where(np.diff(rel) < 0)[0]
    bnd = int(dec[0]) + 1 if len(dec) else len(rel)
    return rel[:bnd], rel[bnd:]


def _pad32(n):
    return ((n + 31) // 32) * 32


def _plan(ng_dt, ng_kl):
    """Static per-call sizes: 4 dt quarters (cell counts) + 1 kl call
    (token count), each %32 (idx stream slices must stay 4B-aligned)."""
    nd_cells = ng_dt // 2                    # dt pairs -> cells
    nd_pad = _pad32(nd_cells)
    q0 = max(32, _pad32(int(nd_cells * 0.12)))
    rem = nd_pad - q0
    q1 = max(32, ((rem // 3) // 32) * 32)
    sizes = [q0, q1, q1, rem - 2 * q1]
    if min(sizes) < 32:                      # tiny dt part: equal-ish split
        base = (nd_pad // _DT_Q) // 32 * 32
        sizes = [base] * (_DT_Q - 1) + [nd_pad - base * (_DT_Q - 1)]
    assert sum(sizes) == nd_pad and all(s % 32 == 0 for s in sizes), sizes
    bounds = np.cumsum([0] + sizes)
    dt_qs = [(int(bounds[i]), int(bounds[i + 1])) for i in range(_DT_Q)]
    nk_pad = _pad32(ng_kl)
    return dt_qs, nd_cells, nd_pad, nk_pad


def _wrap_idxs(vals, num_pad):
    """Pack indices into the ap_gather layout: int16 [128, num_pad//16],
    index j at partition j%16, slot j//16, replicated across 8 Q7 groups."""
    padded = np.zeros(num_pad, np.int16)
    padded[: len(vals)] = vals
    wrapped = padded.reshape(num_pad // 16, 16).T  # [16, S]
    return np.tile(wrapped, (8, 1)).copy()  # [128, S]


def _make_idx_input(rel, _num_idxs=None):
    """idxs input [128, S_total] int16: per-call 16-partition wraps for the
    4 dt quarters (cell indices) then the kl call (token indices)."""
    dt, kl = _split_dt_kl(np.asarray(rel))
    dt_qs, nd_cells, nd_pad, nk_pad = _plan(len(dt), len(kl))
    cells = dt[0::2] // 2
    cols = []
    for lo, hi in dt_qs:
        cols.append(_wrap_idxs(cells[lo:min(hi, nd_cells)], hi - lo))
    cols.append(_wrap_idxs(kl, nk_pad))
    return np.concatenate(cols, axis=1)


def _source_bounds(rels, nelems, ng=None):
    """Per gather call: how many load sub-chunks its source values are
    guaranteed to stay within (max over batches).  Returns 5 bounds:
    4 dt quarters + kl."""
    E = (nelems + _N_SUB - 1) // _N_SUB
    dts, kls = zip(*[_split_dt_kl(np.asarray(r)) for r in rels])
    dt_qs, nd_cells, _, _ = _plan(len(dts[0]), len(kls[0]))
    bounds = []
    for lo, hi in dt_qs:
        vmax = max(int(dt[min(hi, nd_cells) * 2 - 1]) for dt in dts)
        bounds.append(min(_N_SUB, max(1, -(-(vmax + 1) // E))))
    vmax = max(int(kl[-1]) for kl in kls)
    bounds.append(min(_N_SUB, max(1, -(-(vmax + 1) // E))))
    return tuple(bounds)


# ---------------------------------------------------------------------------
# Bass program
# ---------------------------------------------------------------------------

_prog_cache = {}


def _build_program(C, N, start, ng_dt, ng_kl, M, n_iters, nsub,
                   parts=("copy", "load", "gather", "store")):
    """One batch per core: input [C, N] -> output [C, M].

    `nsub[i]` is the number of load sub-chunks gather call i's source
    values stay within (host-verified upper bound; baked into the wait
    structure only).

    `parts` selects pipeline stages (component benchmarking): subset of
    {copy, load, gather, store}; gather needs load, store needs gather."""
    import os
    key = (C, N, start, ng_dt, ng_kl, M, n_iters, tuple(nsub), tuple(parts),
           os.environ.get("COPY_CHUNKS", "2"))
    if key in _prog_cache:
        return _prog_cache[key]
    do_copy = "copy" in parts
    do_load = "load" in parts
    do_gather = "gather" in parts and do_load
    do_store = "store" in parts and do_gather
    do_stest = "stest" in parts    # bench probe: stores alone (garbage data)
    do_stest2 = "stest2" in parts  # probe: one big store (27KB row segments)
    do_ctest = "ctest" in parts    # probe: prefix copy bounced through SBUF

    assert C == 256, "kernel assumes two 128-partition C chunks"
    nelems = N - start
    assert 0 < nelems * 2 <= 2 ** 15, nelems
    E = (nelems + _N_SUB - 1) // _N_SUB
    subs = [(e * E, min(nelems, (e + 1) * E)) for e in range(_N_SUB)]
    dt_qs, nd_cells, nd_pad, nk_pad = _plan(ng_dt, ng_kl)
    NCALL = _DT_Q + 1
    assert len(nsub) == NCALL and all(1 <= n <= _N_SUB for n in nsub), nsub
    # real (unpadded) extent of each call's output, in tokens
    call_tok = []
    for lo, hi in dt_qs:
        call_tok.append((lo * 2, (min(hi, nd_cells) - lo) * 2))
    call_tok.append((ng_dt, ng_kl))          # kl region follows dt region
    ndmax = max(n for _, n in call_tok)
    # idx stream column layout (int16, 16-partition wraps)
    s_off = [0]
    for lo, hi in dt_qs:
        s_off.append(s_off[-1] + (hi - lo) // 16)
    S_total = s_off[-1] + nk_pad // 16
    FO = nd_pad * 4                          # ogI flat: dt cells then kl

    nc = bass.Bass("TRN2")
    inp_t = nc.dram_tensor("input", [C, N], f32, kind="ExternalInput")
    idxs_t = nc.dram_tensor("idxs", [128, S_total], i16, kind="ExternalInput")
    out_t = nc.dram_tensor("output", [C, M], f32, kind="ExternalOutput")
    inp, idxs, out = inp_t.ap(), idxs_t.ap(), out_t.ap()
    # output rows viewed as [row 0:128, plane, col]
    out_pl = out.rearrange("(pl p) m -> p pl m", pl=2)

    stga = [nc.alloc_sbuf_tensor(f"stga{i}", [128, E], f32).ap() for i in range(2)]
    stgb = [nc.alloc_sbuf_tensor(f"stgb{i}", [128, E], f32).ap() for i in range(2)]
    srcI = nc.alloc_sbuf_tensor("srcI", [128, nelems, 2], f32).ap()
    # gather-output windows: call g writes ogW[g%2], de-interleaved into
    # the full-region ogDe which one big store per iteration drains
    # (27KB row segments store at ~473 GB/s vs ~295 for per-call stores)
    wmax = max(max((hi - lo) * 4 for lo, hi in dt_qs), nk_pad * 2)
    ogW = [nc.alloc_sbuf_tensor(f"ogW{i}", [128, wmax], f32).ap()
           for i in range(2)]
    ogDe = nc.alloc_sbuf_tensor("ogDe", [128, 2, ng_dt + ng_kl], f32).ap()
    idxt = nc.alloc_sbuf_tensor("idxt", [128, S_total], i16).ap()

    # probe views: srcI reinterpreted as plane-major scratch
    srcF = srcI.rearrange("p c d -> p (c d)")
    inp_pl = inp.rearrange("(pl p) n -> p pl n", pl=2)

    # cell view of the interleaved source: cell c = tokens (2c, 2c+1)
    cellv = srcI.rearrange("p (c t) d -> p c (t d)", t=2)
    ogW4 = [w.rearrange("p (c u) -> p c u", u=4) for w in ogW]
    ogW2 = [w.rearrange("p (c u) -> p c u", u=2) for w in ogW]
    # de-interleave read views: [p, plane, token]
    deW = [w.rearrange("p (c pl) -> p pl c", pl=2) for w in ogW]

    K = n_iters
    from contextlib import ExitStack

    with ExitStack() as ctx:
        block = ctx.enter_context(nc.Block())
        sL = [ctx.enter_context(nc.semaphore(f"sL{i}")) for i in range(2)]
        sC = ctx.enter_context(nc.semaphore("sC"))     # prefix copies (+16)
        sI = ctx.enter_context(nc.semaphore("sI"))     # idx load (+16)
        sS = [ctx.enter_context(nc.semaphore(f"sS{i}")) for i in range(2)]
        vI = ctx.enter_context(nc.semaphore("vI"))     # interleaves (+1)
        vD = ctx.enter_context(nc.semaphore("vD"))     # de-interleaves (+1)
        gp = ctx.enter_context(nc.semaphore("gp"))     # gathers (+1)

        import os
        ncopy = int(os.environ.get("COPY_CHUNKS", "2"))
        cw = -(-start // ncopy)
        copies = [(c * cw, min(start, (c + 1) * cw)) for c in range(ncopy)]

        # ctest probe layout: prefix copy via SBUF bounce, reads on sync
        # ring / writes on scalar ring, ping-pong [128, 2, ccw] buffers
        # carved out of srcI scratch
        ncc = 5
        ccw = -(-start // ncc)
        cch = [(c * ccw, min(start, (c + 1) * ccw)) for c in range(ncc)]
        cbv = [srcF[:, 2 * ccw * i : 2 * ccw * (i + 1)]
               .rearrange("p (pl j) -> p pl j", pl=2) for i in range(2)]

        @block.sync
        def _(sync):
            if do_ctest:
                ln = [0, 0]
                for k in range(K):
                    for c, (lo, hi) in enumerate(cch):
                        b = c % 2
                        if ln[b] > 0:
                            # buffer reuse: its previous write drained
                            sync.wait_ge(sS[b], 16 * ln[b])
                        sync.dma_start(
                            out=cbv[b][:, :, 0 : hi - lo],
                            in_=inp_pl[:, :, lo:hi],
                        ).then_inc(sL[b], 16)
                        ln[b] += 1
                for b in range(2):
                    sync.wait_ge(sL[b], 16 * ln[b])
                return
            for k in range(K):
                if do_load and do_store and k > 0:
                    # pure-read phase: don't start this iteration's loads
                    # until the previous store (a pure write) has drained
                    sync.wait_ge(sS[0], 16 * k)
                if do_load:
                    for e, (lo, hi) in enumerate(subs):
                        if do_gather:
                            # slot e%2 reused from chunk e-2: its two
                            # interleaves must be done
                            w = 16 * k + 2 * (e - 1)
                            if w > 0:
                                sync.wait_ge(vI, w)
                            # self-wait (race-detector hygiene: orders the
                            # slot sem's next updates)
                            sync.wait_ge(sL[e % 2], 32 * (k * 4 + e // 2))
                        sync.dma_start(
                            out=stga[e % 2][:, 0 : hi - lo],
                            in_=inp[0:128, start + lo : start + hi],
                        ).then_inc(sL[e % 2], 16)
                        sync.dma_start(
                            out=stgb[e % 2][:, 0 : hi - lo],
                            in_=inp[128:256, start + lo : start + hi],
                        ).then_inc(sL[e % 2], 16)
                if do_copy:
                    for lo, hi in copies:
                        sync.dma_start(
                            out=out[:, lo:hi], in_=inp[:, lo:hi]
                        ).then_inc(sC, 16)
            if do_copy:
                sync.wait_ge(sC, 16 * ncopy * K)
            if do_load:
                sync.wait_ge(sL[0], 32 * 4 * K)
                sync.wait_ge(sL[1], 32 * 4 * K)

        @block.vector
        def _(vec):
            if not do_gather:
                return
            for k in range(K):
                for e, (lo, hi) in enumerate(subs):
                    # both loads of this chunk landed
                    vec.wait_ge(sL[e % 2], 32 * (k * 4 + e // 2 + 1))
                    if e == 0 and k > 0:
                        # srcI overwrite: all gathers of iter k-1 done
                        vec.wait_ge(gp, NCALL * k)
                    vec.tensor_copy(
                        srcI[:, lo:hi, 0], stga[e % 2][:, 0 : hi - lo]
                    ).then_inc(vI, 1)
                    vec.tensor_copy(
                        srcI[:, lo:hi, 1], stgb[e % 2][:, 0 : hi - lo]
                    ).then_inc(vI, 1)
                for q in range(NCALL):
                    g = NCALL * k + q
                    vec.wait_ge(gp, g + 1)           # gather (k, q) done
                    if do_store and q == 0 and k > 0:
                        # ogDe overwrite: iter k-1's store drained
                        vec.wait_ge(sS[0], 16 * k)
                    col0, n = call_tok[q]
                    vec.tensor_copy(
                        ogDe[:, :, col0 : col0 + n], deW[g % 2][:, :, 0:n]
                    ).then_inc(vD, 1)

        @block.scalar
        def _(scalar):
            if do_ctest:
                ln = [0, 0]
                for k in range(K):
                    for c, (lo, hi) in enumerate(cch):
                        b = c % 2
                        ln[b] += 1
                        scalar.wait_ge(sL[b], 16 * ln[b])
                        scalar.dma_start(
                            out=out_pl[:, :, lo:hi],
                            in_=cbv[b][:, :, 0 : hi - lo],
                        ).then_inc(sS[b], 16)
                for b in range(2):
                    scalar.wait_ge(sS[b], 16 * ln[b])
                return
            if do_stest2:
                big = srcF[:, 0 : 2 * (ng_dt + ng_kl)].rearrange(
                    "p (pl j) -> p pl j", pl=2)
                for k in range(K):
                    if k > 0:
                        scalar.wait_ge(sS[0], 16 * k)
                    scalar.dma_start(
                        out=out_pl[:, :, start : start + ng_dt + ng_kl],
                        in_=big,
                    ).then_inc(sS[0], 16)
                scalar.wait_ge(sS[0], 16 * K)
                return
            if do_gather:
                scalar.dma_start(out=idxt[:], in_=idxs[:]).then_inc(sI, 16)
            if do_store:
                ngall = ng_dt + ng_kl
                for k in range(K):
                    scalar.wait_ge(vD, NCALL * (k + 1))
                    if do_copy:
                        # keep the store OUT of the copy window: its 7MB
                        # then drains at the pure-write ~473 GB/s instead
                        # of riding the mixed r+w bus at ~340
                        scalar.wait_ge(sC, 16 * ncopy * (k + 1))
                    scalar.dma_start(
                        out=out_pl[:, :, start : start + ngall],
                        in_=ogDe[:],
                    ).then_inc(sS[0], 16)
                scalar.wait_ge(sS[0], 16 * K)

        @block.gpsimd
        def _(g):
            if not do_gather:
                return
            g.load_library(library_config.ap_gather)
            g.wait_ge(sI, 16)
            for k in range(K):
                for q in range(NCALL):
                    gidx = NCALL * k + q
                    g.wait_ge(vI, 16 * k + 2 * nsub[q])
                    if gidx >= 2:
                        # window gidx%2 reuse: deint of call gidx-2 done
                        g.wait_ge(vD, gidx - 1)
                    bq = min(nelems, nsub[q] * E)
                    b = gidx % 2
                    if q < _DT_Q:
                        lo, hi = dt_qs[q]
                        bqc = min(nelems, bq + 1) // 2
                        g.ap_gather(
                            out_ap=ogW4[b][:, 0 : hi - lo, :],
                            in_ap=cellv[:, 0:bqc, :],
                            idxs_ap=idxt[:, s_off[q] : s_off[q + 1]],
                            channels=128,
                            num_elems=bqc,
                            d=4,
                            num_idxs=hi - lo,
                        ).then_inc(gp, 1)
                    else:
                        g.ap_gather(
                            out_ap=ogW2[b][:, 0:nk_pad, :],
                            in_ap=srcI[:, 0:bq, :],
                            idxs_ap=idxt[:, s_off[q] : s_off[q] + nk_pad // 16],
                            channels=128,
                            num_elems=bq,
                            d=2,
                            num_idxs=nk_pad,
                        ).then_inc(gp, 1)

    # Populate .instr bytes for extended-inst InstISA subclasses (APGather,
    # PseudoReloadLibraryIndex). Raw Bass doesn't run this pass; without it
    # walrus fails with "ISA wrong length".
    mybir.codegen_inst_isa_subclasses(nc)

    _prog_cache[key] = (nc, None)
    return nc, None


# ---------------------------------------------------------------------------
# Public entry point
# ---------------------------------------------------------------------------

def _program_and_inputs(x, mask, n_iters=1,
                        parts=("copy", "load", "gather", "store")):
    """Build (nc, in_maps) for the device path (test/bench helper)."""
    B, C, N = x.shape
    per_batch = [_batch_indices(mask[b, 0]) for b in range(B)]
    start0 = per_batch[0][0]
    rels = [r for _, r, _ in per_batch]
    splits = [_split_dt_kl(r) for r in rels]
    nd0, nk0 = len(splits[0][0]), len(splits[0][1])
    M = max(s + len(r) for s, r, _ in per_batch)
    nsub = _source_bounds(rels, N - start0)
    nc, _ = _build_program(C, N, start0, nd0, nk0, M, n_iters, nsub,
                           parts=parts)
    in_maps = [
        {
            "input": np.ascontiguousarray(x[b]),
            "idxs": _make_idx_input(rels[b]),
        }
        for b in range(B)
    ]
    return nc, in_maps


def kernel(input, pooling_mask, _n_iters=1):
    x = np.asarray(input)
    mask = np.asarray(pooling_mask)
    B, C, N = x.shape
    assert x.dtype == np.float32

    per_batch = [_batch_indices(mask[b, 0]) for b in range(B)]
    M = max(s + len(r) for s, r, _ in per_batch)
    device_ok = (
        B == _NUM_CORES
        and C == 256
        and len({s for s, _, _ in per_batch}) == 1
        and len({len(r) for _, r, _ in per_batch}) == 1
    )
    start0 = per_batch[0][0]
    if device_ok:
        splits = [_split_dt_kl(r) for _, r, _ in per_batch]
        nd0, nk0 = len(splits[0][0]), len(splits[0][1])
        device_ok = (
            0 < (N - start0) * 2 <= 2 ** 15
            and nd0 > 8 * 32 * 2        # 4 quarters of >=32 cells
            and nk0 > 0
            and all(len(dt) == nd0 and len(kl) == nk0 for dt, kl in splits)
            # dt part must be even-aligned contiguous pairs
            and all(
                len(dt) % 2 == 0
                and (dt[0::2] % 2 == 0).all()
                and (dt[1::2] == dt[0::2] + 1).all()
                for dt, _ in splits
            )
        )
    if not device_ok:
        # Irregular structure (not produced by this module's mask builder)
        # - fall back to a host gather.
        out = np.zeros((B, C, M), np.float32)
        for b, (s, rel, _) in enumerate(per_batch):
            n = s + len(rel)
            g = np.concatenate([np.arange(s, dtype=np.int64), s + rel])
            out[b, :, :n] = x[b][:, g]
        return out

    rels = [r for _, r, _ in per_batch]
    nsub = _source_bounds(rels, N - start0)
    nc, _ = _build_program(C, N, start0, nd0, nk0, M, _n_iters, nsub)
    in_maps = [
        {
            "input": np.ascontiguousarray(x[b]),
            "idxs": _make_idx_input(rels[b]),
        }
        for b in range(B)
    ]
    res = run_bass_kernel_spmd(nc, in_maps, list(range(_NUM_CORES)))
    return np.stack([res.results[b]["output"] for b in range(B)])


# revision 27
# speedup vs baseline: 1.0744x; 1.0744x over previous
"""Trainium2 Bass kernel for BatchIrregularDownsample2d (D=2).

Contract: kernel(**inputs) takes the FULL inputs
    input:        [B, C, N]  float32
    pooling_mask: [B, 1, H, W] int32
and returns the FULL output [B, C, M] float32, where M is the max
per-batch compacted length (identical across batches for quadtree masks
with equal level histograms, which is what this module produces).

Strategy (pure data-parallel over B, one batch per NeuronCore):
  The reference gather G[b] splits into
    - an identity prefix  out[:, :start]         = in[:, :start]
    - a gather            out[:, start:start+ng] = in[:, start + rel[j]]
  The gather indices further split into a "dt" part (dont-touch tokens,
  sorted, and - because quadtree blocks contribute aligned 2-token units
  to the compacted stream - consisting entirely of even-aligned pairs
  (2c, 2c+1)) followed by a "kl" part (pooled representatives, sorted
  singles).

  Per core: the prefix is a DRAM->DRAM DMA; the gather source region
  [C=256, nelems] is DMA'd into SBUF in 8 staged chunks which the DVE
  interleaves elementwise into srcI[128, nelems, 2] (both 128-partition
  C-chunks per token).  The GPSIMD ap_gather op costs ~140 cycles per
  4-index request regardless of d, so the dt part is gathered with d=4
  over an aligned 2-token cell view of srcI (cell index c = pair (2c,
  2c+1); half the requests of a d=2 token gather), and the kl part with
  d=2 over the token view.  The DVE de-interleaves each gathered range
  into [2, n] plane-major bounce buffers that a single DMA per range
  stores to both 128-row halves of the output.  Index arithmetic is
  host-side numpy (as in the original torch module, which syncs the mask
  to host anyway).
"""

import numpy as np

from concourse import bass, library_config, mybir
from concourse.bass_utils import run_bass_kernel_spmd

f32 = mybir.dt.float32
i16 = mybir.dt.int16

_NUM_CORES = 8
_N_SUB = 8          # gather-region load sub-chunks
_DT_Q = 4           # positional sub-gathers over the dt cell stream


# ---------------------------------------------------------------------------
# Host-side index computation (replicates reference._build_indices, D=2)
# ---------------------------------------------------------------------------

def _batch_indices(mask2d):
    """mask2d: [H, W] int32 quadtree mask. Returns (start, rel_idx int64[ng])
    with absolute gather index = start + rel_idx."""
    D = 2
    s = 2 ** (D - 1)
    start = 0
    for i in range(D - 1):
        start += int((mask2d == i).sum()) // (4 ** i)
    cs = (mask2d >= D - 1)[::s, ::s]
    dt = (mask2d < D)[::s, ::s]
    r, c = np.nonzero(cs)
    topleft = ((r % 2) + (c % 2)) == 0
    dt_at = dt[r, c]
    keep_lower = topleft & ~dt_at
    pos = np.arange(r.shape[0])
    rel = np.concatenate([pos[dt_at], pos[keep_lower]]).astype(np.int64)
    return start, rel, int(r.shape[0])


def _split_dt_kl(rel):
    """rel = [dt part (sorted), kl part (sorted)]; returns (dt, kl)."""
    dec = np.where(np.diff(rel) < 0)[0]
    bnd = int(dec[0]) + 1 if len(dec) else len(rel)
    return rel[:bnd], rel[bnd:]


def _pad32(n):
    return ((n + 31) // 32) * 32


def _plan(ng_dt, ng_kl):
    """Static per-call sizes: 4 dt quarters (cell counts) + 1 kl call
    (token count), each %32 (idx stream slices must stay 4B-aligned)."""
    nd_cells = ng_dt // 2                    # dt pairs -> cells
    nd_pad = _pad32(nd_cells)
    q0 = max(32, _pad32(int(nd_cells * 0.12)))
    rem = nd_pad - q0
    q1 = max(32, ((rem // 3) // 32) * 32)
    sizes = [q0, q1, q1, rem - 2 * q1]
    if min(sizes) < 32:                      # tiny dt part: equal-ish split
        base = (nd_pad // _DT_Q) // 32 * 32
        sizes = [base] * (_DT_Q - 1) + [nd_pad - base * (_DT_Q - 1)]
    assert sum(sizes) == nd_pad and all(s % 32 == 0 for s in sizes), sizes
    bounds = np.cumsum([0] + sizes)
    dt_qs = [(int(bounds[i]), int(bounds[i + 1])) for i in range(_DT_Q)]
    nk_pad = _pad32(ng_kl)
    return dt_qs, nd_cells, nd_pad, nk_pad


def _wrap_idxs(vals, num_pad):
    """Pack indices into the ap_gather layout: int16 [128, num_pad//16],
    index j at partition j%16, slot j//16, replicated across 8 Q7 groups."""
    padded = np.zeros(num_pad, np.int16)
    padded[: len(vals)] = vals
    wrapped = padded.reshape(num_pad // 16, 16).T  # [16, S]
    return np.tile(wrapped, (8, 1)).copy()  # [128, S]


def _make_idx_input(rel, _num_idxs=None):
    """idxs input [128, S_total] int16: per-call 16-partition wraps for the
    4 dt quarters (cell indices) then the kl call (token indices)."""
    dt, kl = _split_dt_kl(np.asarray(rel))
    dt_qs, nd_cells, nd_pad, nk_pad = _plan(len(dt), len(kl))
    cells = dt[0::2] // 2
    cols = []
    for lo, hi in dt_qs:
        cols.append(_wrap_idxs(cells[lo:min(hi, nd_cells)], hi - lo))
    cols.append(_wrap_idxs(kl, nk_pad))
    return np.concatenate(cols, axis=1)


def _source_bounds(rels, nelems, ng=None):
    """Per gather call: how many load sub-chunks its source values are
    guaranteed to stay within (max over batches).  Returns 5 bounds:
    4 dt quarters + kl."""
    E = (nelems + _N_SUB - 1) // _N_SUB
    dts, kls = zip(*[_split_dt_kl(np.asarray(r)) for r in rels])
    dt_qs, nd_cells, _, _ = _plan(len(dts[0]), len(kls[0]))
    bounds = []
    for lo, hi in dt_qs:
        vmax = max(int(dt[min(hi, nd_cells) * 2 - 1]) for dt in dts)
        bounds.append(min(_N_SUB, max(1, -(-(vmax + 1) // E))))
    vmax = max(int(kl[-1]) for kl in kls)
    bounds.append(min(_N_SUB, max(1, -(-(vmax + 1) // E))))
    return tuple(bounds)


# ---------------------------------------------------------------------------
# Bass program
# ---------------------------------------------------------------------------

_prog_cache = {}


def _build_program(C, N, start, ng_dt, ng_kl, M, n_iters, nsub,
                   parts=("copy", "load", "gather", "store")):
    """One batch per core: input [C, N] -> output [C, M].

    `nsub[i]` is the number of load sub-chunks gather call i's source
    values stay within (host-verified upper bound; baked into the wait
    structure only).

    `parts` selects pipeline stages (component benchmarking): subset of
    {copy, load, gather, store}; gather needs load, store needs gather."""
    import os
    key = (C, N, start, ng_dt, ng_kl, M, n_iters, tuple(nsub), tuple(parts),
           os.environ.get("COPY_CHUNKS", "2"))
    if key in _prog_cache:
        return _prog_cache[key]
    do_copy = "copy" in parts
    do_load = "load" in parts
    do_gather = "gather" in parts and do_load
    do_store = "store" in parts and do_gather
    do_stest = "stest" in parts    # bench probe: stores alone (garbage data)
    do_stest2 = "stest2" in parts  # probe: one big store (27KB row segments)
    do_ctest = "ctest" in parts    # probe: prefix copy bounced through SBUF

    assert C == 256, "kernel assumes two 128-partition C chunks"
    nelems = N - start
    assert 0 < nelems * 2 <= 2 ** 15, nelems
    E = (nelems + _N_SUB - 1) // _N_SUB
    subs = [(e * E, min(nelems, (e + 1) * E)) for e in range(_N_SUB)]
    dt_qs, nd_cells, nd_pad, nk_pad = _plan(ng_dt, ng_kl)
    NCALL = _DT_Q + 1
    assert len(nsub) == NCALL and all(1 <= n <= _N_SUB for n in nsub), nsub
    # real (unpadded) extent of each call's output, in tokens
    call_tok = []
    for lo, hi in dt_qs:
        call_tok.append((lo * 2, (min(hi, nd_cells) - lo) * 2))
    call_tok.append((ng_dt, ng_kl))          # kl region follows dt region
    ndmax = max(n for _, n in call_tok)
    # idx stream column layout (int16, 16-partition wraps)
    s_off = [0]
    for lo, hi in dt_qs:
        s_off.append(s_off[-1] + (hi - lo) // 16)
    S_total = s_off[-1] + nk_pad // 16
    FO = nd_pad * 4                          # ogI flat: dt cells then kl

    nc = bass.Bass("TRN2")
    inp_t = nc.dram_tensor("input", [C, N], f32, kind="ExternalInput")
    idxs_t = nc.dram_tensor("idxs", [128, S_total], i16, kind="ExternalInput")
    out_t = nc.dram_tensor("output", [C, M], f32, kind="ExternalOutput")
    inp, idxs, out = inp_t.ap(), idxs_t.ap(), out_t.ap()
    # output rows viewed as [row 0:128, plane, col]
    out_pl = out.rearrange("(pl p) m -> p pl m", pl=2)

    stga = [nc.alloc_sbuf_tensor(f"stga{i}", [128, E], f32).ap() for i in range(2)]
    stgb = [nc.alloc_sbuf_tensor(f"stgb{i}", [128, E], f32).ap() for i in range(2)]
    srcI = nc.alloc_sbuf_tensor("srcI", [128, nelems, 2], f32).ap()
    # gather-output windows: call g writes ogW[g%2], de-interleaved into
    # the full-region ogDe which one big store per iteration drains
    # (27KB row segments store at ~473 GB/s vs ~295 for per-call stores)
    wmax = max(max((hi - lo) * 4 for lo, hi in dt_qs), nk_pad * 2)
    ogW = [nc.alloc_sbuf_tensor(f"ogW{i}", [128, wmax], f32).ap()
           for i in range(2)]
    ogDe = nc.alloc_sbuf_tensor("ogDe", [128, 2, ng_dt + ng_kl], f32).ap()
    idxt = nc.alloc_sbuf_tensor("idxt", [128, S_total], i16).ap()

    # probe views: srcI reinterpreted as plane-major scratch
    srcF = srcI.rearrange("p c d -> p (c d)")
    inp_pl = inp.rearrange("(pl p) n -> p pl n", pl=2)

    # cell view of the interleaved source: cell c = tokens (2c, 2c+1)
    cellv = srcI.rearrange("p (c t) d -> p c (t d)", t=2)
    ogW4 = [w.rearrange("p (c u) -> p c u", u=4) for w in ogW]
    ogW2 = [w.rearrange("p (c u) -> p c u", u=2) for w in ogW]
    # de-interleave read views: [p, plane, token]
    deW = [w.rearrange("p (c pl) -> p pl c", pl=2) for w in ogW]

    K = n_iters
    from contextlib import ExitStack

    with ExitStack() as ctx:
        block = ctx.enter_context(nc.Block())
        sL = [ctx.enter_context(nc.semaphore(f"sL{i}")) for i in range(2)]
        sC = ctx.enter_context(nc.semaphore("sC"))     # prefix copies (+16)
        sI = ctx.enter_context(nc.semaphore("sI"))     # idx load (+16)
        sS = [ctx.enter_context(nc.semaphore(f"sS{i}")) for i in range(2)]
        vI = ctx.enter_context(nc.semaphore("vI"))     # interleaves (+1)
        vD = ctx.enter_context(nc.semaphore("vD"))     # de-interleaves (+1)
        gp = ctx.enter_context(nc.semaphore("gp"))     # gathers (+1)

        import os
        ncopy = int(os.environ.get("COPY_CHUNKS", "2"))
        cw = -(-start // ncopy)
        copies = [(c * cw, min(start, (c + 1) * cw)) for c in range(ncopy)]

        # ctest probe layout: prefix copy via SBUF bounce, reads on sync
        # ring / writes on scalar ring, ping-pong [128, 2, ccw] buffers
        # carved out of srcI scratch
        ncc = 5
        ccw = -(-start // ncc)
        cch = [(c * ccw, min(start, (c + 1) * ccw)) for c in range(ncc)]
        cbv = [srcF[:, 2 * ccw * i : 2 * ccw * (i + 1)]
               .rearrange("p (pl j) -> p pl j", pl=2) for i in range(2)]

        @block.sync
        def _(sync):
            if do_ctest:
                ln = [0, 0]
                for k in range(K):
                    for c, (lo, hi) in enumerate(cch):
                        b = c % 2
                        if ln[b] > 0:
                            # buffer reuse: its previous write drained
                            sync.wait_ge(sS[b], 16 * ln[b])
                        sync.dma_start(
                            out=cbv[b][:, :, 0 : hi - lo],
                            in_=inp_pl[:, :, lo:hi],
                        ).then_inc(sL[b], 16)
                        ln[b] += 1
                for b in range(2):
                    sync.wait_ge(sL[b], 16 * ln[b])
                return
            for k in range(K):
                if do_load:
                    for e, (lo, hi) in enumerate(subs):
                        if do_gather:
                            # slot e%2 reused from chunk e-2: its two
                            # interleaves must be done
                            w = 16 * k + 2 * (e - 1)
                            if w > 0:
                                sync.wait_ge(vI, w)
                            # self-wait (race-detector hygiene: orders the
                            # slot sem's next updates)
                            sync.wait_ge(sL[e % 2], 32 * (k * 4 + e // 2))
                        sync.dma_start(
                            out=stga[e % 2][:, 0 : hi - lo],
                            in_=inp[0:128, start + lo : start + hi],
                        ).then_inc(sL[e % 2], 16)
                        sync.dma_start(
                            out=stgb[e % 2][:, 0 : hi - lo],
                            in_=inp[128:256, start + lo : start + hi],
                        ).then_inc(sL[e % 2], 16)
                if do_copy:
                    for lo, hi in copies:
                        sync.dma_start(
                            out=out[:, lo:hi], in_=inp[:, lo:hi]
                        ).then_inc(sC, 16)
            if do_copy:
                sync.wait_ge(sC, 16 * ncopy * K)
            if do_load:
                sync.wait_ge(sL[0], 32 * 4 * K)
                sync.wait_ge(sL[1], 32 * 4 * K)

        @block.vector
        def _(vec):
            if not do_gather:
                return
            for k in range(K):
                for e, (lo, hi) in enumerate(subs):
                    # both loads of this chunk landed
                    vec.wait_ge(sL[e % 2], 32 * (k * 4 + e // 2 + 1))
                    if e == 0 and k > 0:
                        # srcI overwrite: all gathers of iter k-1 done
                        vec.wait_ge(gp, NCALL * k)
                    vec.tensor_copy(
                        srcI[:, lo:hi, 0], stga[e % 2][:, 0 : hi - lo]
                    ).then_inc(vI, 1)
                    vec.tensor_copy(
                        srcI[:, lo:hi, 1], stgb[e % 2][:, 0 : hi - lo]
                    ).then_inc(vI, 1)
                for q in range(NCALL):
                    g = NCALL * k + q
                    vec.wait_ge(gp, g + 1)           # gather (k, q) done
                    if do_store and q == 0 and k > 0:
                        # ogDe overwrite: iter k-1's store drained
                        vec.wait_ge(sS[0], 16 * k)
                    col0, n = call_tok[q]
                    vec.tensor_copy(
                        ogDe[:, :, col0 : col0 + n], deW[g % 2][:, :, 0:n]
                    ).then_inc(vD, 1)

        @block.scalar
        def _(scalar):
            if do_ctest:
                ln = [0, 0]
                for k in range(K):
                    for c, (lo, hi) in enumerate(cch):
                        b = c % 2
                        ln[b] += 1
                        scalar.wait_ge(sL[b], 16 * ln[b])
                        scalar.dma_start(
                            out=out_pl[:, :, lo:hi],
                            in_=cbv[b][:, :, 0 : hi - lo],
                        ).then_inc(sS[b], 16)
                for b in range(2):
                    scalar.wait_ge(sS[b], 16 * ln[b])
                return
            if do_stest2:
                big = srcF[:, 0 : 2 * (ng_dt + ng_kl)].rearrange(
                    "p (pl j) -> p pl j", pl=2)
                for k in range(K):
                    if k > 0:
                        scalar.wait_ge(sS[0], 16 * k)
                    scalar.dma_start(
                        out=out_pl[:, :, start : start + ng_dt + ng_kl],
                        in_=big,
                    ).then_inc(sS[0], 16)
                scalar.wait_ge(sS[0], 16 * K)
                return
            if do_gather:
                scalar.dma_start(out=idxt[:], in_=idxs[:]).then_inc(sI, 16)
            if do_store:
                ngall = ng_dt + ng_kl
                for k in range(K):
                    scalar.wait_ge(vD, NCALL * (k + 1))
                    scalar.dma_start(
                        out=out_pl[:, :, start : start + ngall],
                        in_=ogDe[:],
                    ).then_inc(sS[0], 16)
                scalar.wait_ge(sS[0], 16 * K)

        @block.gpsimd
        def _(g):
            if not do_gather:
                return
            g.load_library(library_config.ap_gather)
            g.wait_ge(sI, 16)
            for k in range(K):
                for q in range(NCALL):
                    gidx = NCALL * k + q
                    g.wait_ge(vI, 16 * k + 2 * nsub[q])
                    if gidx >= 2:
                        # window gidx%2 reuse: deint of call gidx-2 done
                        g.wait_ge(vD, gidx - 1)
                    bq = min(nelems, nsub[q] * E)
                    b = gidx % 2
                    if q < _DT_Q:
                        lo, hi = dt_qs[q]
                        bqc = min(nelems, bq + 1) // 2
                        g.ap_gather(
                            out_ap=ogW4[b][:, 0 : hi - lo, :],
                            in_ap=cellv[:, 0:bqc, :],
                            idxs_ap=idxt[:, s_off[q] : s_off[q + 1]],
                            channels=128,
                            num_elems=bqc,
                            d=4,
                            num_idxs=hi - lo,
                        ).then_inc(gp, 1)
                    else:
                        g.ap_gather(
                            out_ap=ogW2[b][:, 0:nk_pad, :],
                            in_ap=srcI[:, 0:bq, :],
                            idxs_ap=idxt[:, s_off[q] : s_off[q] + nk_pad // 16],
                            channels=128,
                            num_elems=bq,
                            d=2,
                            num_idxs=nk_pad,
                        ).then_inc(gp, 1)

    # Populate .instr bytes for extended-inst InstISA subclasses (APGather,
    # PseudoReloadLibraryIndex). Raw Bass doesn't run this pass; without it
    # walrus fails with "ISA wrong length".
    mybir.codegen_inst_isa_subclasses(nc)

    _prog_cache[key] = (nc, None)
    return nc, None


# ---------------------------------------------------------------------------
# Public entry point
# ---------------------------------------------------------------------------

def _program_and_inputs(x, mask, n_iters=1,
                        parts=("copy", "load", "gather", "store")):
    """Build (nc, in_maps) for the device path (test/bench helper)."""
    B, C, N = x.shape
    per_batch = [_batch_indices(mask[b, 0]) for b in range(B)]
    start0 = per_batch[0][0]
    rels = [r for _, r, _ in per_batch]
    splits = [_split_dt_kl(r) for r in rels]
    nd0, nk0 = len(splits[0][0]), len(splits[0][1])
    M = max(s + len(r) for s, r, _ in per_batch)
    nsub = _source_bounds(rels, N - start0)
    nc, _ = _build_program(C, N, start0, nd0, nk0, M, n_iters, nsub,
                           parts=parts)
    in_maps = [
        {
            "input": np.ascontiguousarray(x[b]),
            "idxs": _make_idx_input(rels[b]),
        }
        for b in range(B)
    ]
    return nc, in_maps


def kernel(input, pooling_mask, _n_iters=1):
    x = np.asarray(input)
    mask = np.asarray(pooling_mask)
    B, C, N = x.shape
    assert x.dtype == np.float32

    per_batch = [_batch_indices(mask[b, 0]) for b in range(B)]
    M = max(s + len(r) for s, r, _ in per_batch)
    device_ok = (
        B == _NUM_CORES
        and C == 256
        and len({s for s, _, _ in per_batch}) == 1
        and len({len(r) for _, r, _ in per_batch}) == 1
    )
    start0 = per_batch[0][0]
    if device_ok:
        splits = [_split_dt_kl(r) for _, r, _ in per_batch]
        nd0, nk0 = len(splits[0][0]), len(splits[0][1])
        device_ok = (
            0 < (N - start0) * 2 <= 2 ** 15
            and nd0 > 8 * 32 * 2        # 4 quarters of >=32 cells
            and nk0 > 0
            and all(len(dt) == nd0 and len(kl) == nk0 for dt, kl in splits)
            # dt part must be even-aligned contiguous pairs
            and all(
                len(dt) % 2 == 0
                and (dt[0::2] % 2 == 0).all()
                and (dt[1::2] == dt[0::2] + 1).all()
                for dt, _ in splits
            )
        )
    if not device_ok:
        # Irregular structure (not produced by this module's mask builder)
        # - fall back to a host gather.
        out = np.zeros((B, C, M), np.float32)
        for b, (s, rel, _) in enumerate(per_batch):
            n = s + len(rel)
            g = np.concatenate([np.arange(s, dtype=np.int64), s + rel])
            out[b, :, :n] = x[b][:, g]
        return out

    rels = [r for _, r, _ in per_batch]
    nsub = _source_bounds(rels, N - start0)
    nc, _ = _build_program(C, N, start0, nd0, nk0, M, _n_iters, nsub)
    in_maps = [
        {
            "input": np.ascontiguousarray(x[b]),
            "idxs": _make_idx_input(rels[b]),
        }
        for b in range(B)
    ]
    res = run_bass_kernel_spmd(nc, in_maps, list(range(_NUM_CORES)))
    return np.stack([res.results[b]["output"] for b in range(B)])


# revision 28
# speedup vs baseline: 1.1752x; 1.0938x over previous
"""Trainium2 Bass kernel for BatchIrregularDownsample2d (D=2).

Contract: kernel(**inputs) takes the FULL inputs
    input:        [B, C, N]  float32
    pooling_mask: [B, 1, H, W] int32
and returns the FULL output [B, C, M] float32, where M is the max
per-batch compacted length (identical across batches for quadtree masks
with equal level histograms, which is what this module produces).

Strategy (pure data-parallel over B, one batch per NeuronCore):
  The reference gather G[b] splits into
    - an identity prefix  out[:, :start]         = in[:, :start]
    - a gather            out[:, start:start+ng] = in[:, start + rel[j]]
  The gather indices further split into a "dt" part (dont-touch tokens,
  sorted, and - because quadtree blocks contribute aligned 2-token units
  to the compacted stream - consisting entirely of even-aligned pairs
  (2c, 2c+1)) followed by a "kl" part (pooled representatives, sorted
  singles).

  Per core: the prefix is a DRAM->DRAM DMA; the gather source region
  [C=256, nelems] is DMA'd into SBUF in 8 staged chunks which the DVE
  interleaves elementwise into srcI[128, nelems, 2] (both 128-partition
  C-chunks per token).  The GPSIMD ap_gather op costs ~140 cycles per
  4-index request regardless of d, so the dt part is gathered with d=4
  over an aligned 2-token cell view of srcI (cell index c = pair (2c,
  2c+1); half the requests of a d=2 token gather), and the kl part with
  d=2 over the token view.  Each gather call writes one of two small
  window buffers; the DVE de-interleaves every gathered range into a
  full-region plane-major bounce buffer [128, 2, ng] that ONE store DMA
  per iteration drains to both 128-row output halves (27 KB row
  segments store at ~473 GB/s vs ~295 for small per-range stores).
  Index arithmetic is host-side numpy (as in the original torch module,
  which syncs the mask to host anyway).
"""

import numpy as np

from concourse import bass, library_config, mybir
from concourse.bass_utils import run_bass_kernel_spmd

f32 = mybir.dt.float32
i16 = mybir.dt.int16

_NUM_CORES = 8
_N_SUB = 8          # gather-region load sub-chunks
_DT_Q = 4           # positional sub-gathers over the dt cell stream


# ---------------------------------------------------------------------------
# Host-side index computation (replicates reference._build_indices, D=2)
# ---------------------------------------------------------------------------

def _batch_indices(mask2d):
    """mask2d: [H, W] int32 quadtree mask. Returns (start, rel_idx int64[ng])
    with absolute gather index = start + rel_idx."""
    D = 2
    s = 2 ** (D - 1)
    start = 0
    for i in range(D - 1):
        start += int((mask2d == i).sum()) // (4 ** i)
    cs = (mask2d >= D - 1)[::s, ::s]
    dt = (mask2d < D)[::s, ::s]
    r, c = np.nonzero(cs)
    topleft = ((r % 2) + (c % 2)) == 0
    dt_at = dt[r, c]
    keep_lower = topleft & ~dt_at
    pos = np.arange(r.shape[0])
    rel = np.concatenate([pos[dt_at], pos[keep_lower]]).astype(np.int64)
    return start, rel, int(r.shape[0])


def _split_dt_kl(rel):
    """rel = [dt part (sorted), kl part (sorted)]; returns (dt, kl)."""
    dec = np.where(np.diff(rel) < 0)[0]
    bnd = int(dec[0]) + 1 if len(dec) else len(rel)
    return rel[:bnd], rel[bnd:]


def _pad32(n):
    return ((n + 31) // 32) * 32


def _plan(ng_dt, ng_kl):
    """Static per-call sizes: 4 dt quarters (cell counts) + 1 kl call
    (token count), each %32 (idx stream slices must stay 4B-aligned)."""
    nd_cells = ng_dt // 2                    # dt pairs -> cells
    nd_pad = _pad32(nd_cells)
    q0 = max(32, _pad32(int(nd_cells * 0.12)))
    rem = nd_pad - q0
    q1 = max(32, ((rem // 3) // 32) * 32)
    sizes = [q0, q1, q1, rem - 2 * q1]
    if min(sizes) < 32:                      # tiny dt part: equal-ish split
        base = (nd_pad // _DT_Q) // 32 * 32
        sizes = [base] * (_DT_Q - 1) + [nd_pad - base * (_DT_Q - 1)]
    assert sum(sizes) == nd_pad and all(s % 32 == 0 for s in sizes), sizes
    bounds = np.cumsum([0] + sizes)
    dt_qs = [(int(bounds[i]), int(bounds[i + 1])) for i in range(_DT_Q)]
    nk_pad = _pad32(ng_kl)
    return dt_qs, nd_cells, nd_pad, nk_pad


def _wrap_idxs(vals, num_pad):
    """Pack indices into the ap_gather layout: int16 [128, num_pad//16],
    index j at partition j%16, slot j//16, replicated across 8 Q7 groups."""
    padded = np.zeros(num_pad, np.int16)
    padded[: len(vals)] = vals
    wrapped = padded.reshape(num_pad // 16, 16).T  # [16, S]
    return np.tile(wrapped, (8, 1)).copy()  # [128, S]


def _make_idx_input(rel, _num_idxs=None):
    """idxs input [128, S_total] int16: per-call 16-partition wraps for the
    4 dt quarters (cell indices) then the kl call (token indices)."""
    dt, kl = _split_dt_kl(np.asarray(rel))
    dt_qs, nd_cells, nd_pad, nk_pad = _plan(len(dt), len(kl))
    cells = dt[0::2] // 2
    cols = []
    for lo, hi in dt_qs:
        cols.append(_wrap_idxs(cells[lo:min(hi, nd_cells)], hi - lo))
    cols.append(_wrap_idxs(kl, nk_pad))
    return np.concatenate(cols, axis=1)


def _source_bounds(rels, nelems, ng=None):
    """Per gather call: how many load sub-chunks its source values are
    guaranteed to stay within (max over batches).  Returns 5 bounds:
    4 dt quarters + kl."""
    E = (nelems + _N_SUB - 1) // _N_SUB
    dts, kls = zip(*[_split_dt_kl(np.asarray(r)) for r in rels])
    dt_qs, nd_cells, _, _ = _plan(len(dts[0]), len(kls[0]))
    bounds = []
    for lo, hi in dt_qs:
        vmax = max(int(dt[min(hi, nd_cells) * 2 - 1]) for dt in dts)
        bounds.append(min(_N_SUB, max(1, -(-(vmax + 1) // E))))
    vmax = max(int(kl[-1]) for kl in kls)
    bounds.append(min(_N_SUB, max(1, -(-(vmax + 1) // E))))
    return tuple(bounds)


# ---------------------------------------------------------------------------
# Bass program
# ---------------------------------------------------------------------------

_prog_cache = {}


def _build_program(C, N, start, ng_dt, ng_kl, M, n_iters, nsub,
                   parts=("copy", "load", "gather", "store")):
    """One batch per core: input [C, N] -> output [C, M].

    `nsub[i]` is the number of load sub-chunks gather call i's source
    values stay within (host-verified upper bound; baked into the wait
    structure only).

    `parts` selects pipeline stages (component benchmarking): subset of
    {copy, load, gather, store}; gather needs load, store needs gather."""
    import os
    key = (C, N, start, ng_dt, ng_kl, M, n_iters, tuple(nsub), tuple(parts),
           os.environ.get("COPY_CHUNKS", "2"))
    if key in _prog_cache:
        return _prog_cache[key]
    do_copy = "copy" in parts
    do_load = "load" in parts
    do_gather = "gather" in parts and do_load
    do_store = "store" in parts and do_gather
    do_stest = "stest" in parts    # bench probe: stores alone (garbage data)
    do_stest2 = "stest2" in parts  # probe: one big store (27KB row segments)
    do_ctest = "ctest" in parts    # probe: prefix copy bounced through SBUF

    assert C == 256, "kernel assumes two 128-partition C chunks"
    nelems = N - start
    assert 0 < nelems * 2 <= 2 ** 15, nelems
    E = (nelems + _N_SUB - 1) // _N_SUB
    subs = [(e * E, min(nelems, (e + 1) * E)) for e in range(_N_SUB)]
    dt_qs, nd_cells, nd_pad, nk_pad = _plan(ng_dt, ng_kl)
    NCALL = _DT_Q + 1
    assert len(nsub) == NCALL and all(1 <= n <= _N_SUB for n in nsub), nsub
    # real (unpadded) extent of each call's output, in tokens
    call_tok = []
    for lo, hi in dt_qs:
        call_tok.append((lo * 2, (min(hi, nd_cells) - lo) * 2))
    call_tok.append((ng_dt, ng_kl))          # kl region follows dt region
    ndmax = max(n for _, n in call_tok)
    # idx stream column layout (int16, 16-partition wraps)
    s_off = [0]
    for lo, hi in dt_qs:
        s_off.append(s_off[-1] + (hi - lo) // 16)
    S_total = s_off[-1] + nk_pad // 16
    FO = nd_pad * 4                          # ogI flat: dt cells then kl

    nc = bass.Bass("TRN2")
    inp_t = nc.dram_tensor("input", [C, N], f32, kind="ExternalInput")
    idxs_t = nc.dram_tensor("idxs", [128, S_total], i16, kind="ExternalInput")
    out_t = nc.dram_tensor("output", [C, M], f32, kind="ExternalOutput")
    inp, idxs, out = inp_t.ap(), idxs_t.ap(), out_t.ap()
    # output rows viewed as [row 0:128, plane, col]
    out_pl = out.rearrange("(pl p) m -> p pl m", pl=2)

    stga = [nc.alloc_sbuf_tensor(f"stga{i}", [128, E], f32).ap() for i in range(2)]
    stgb = [nc.alloc_sbuf_tensor(f"stgb{i}", [128, E], f32).ap() for i in range(2)]
    srcI = nc.alloc_sbuf_tensor("srcI", [128, nelems, 2], f32).ap()
    # gather-output windows: call g writes ogW[g%2], de-interleaved into
    # the full-region ogDe which one big store per iteration drains
    # (27KB row segments store at ~473 GB/s vs ~295 for per-call stores)
    wmax = max(max((hi - lo) * 4 for lo, hi in dt_qs), nk_pad * 2)
    ogW = [nc.alloc_sbuf_tensor(f"ogW{i}", [128, wmax], f32).ap()
           for i in range(2)]
    ogDe = nc.alloc_sbuf_tensor("ogDe", [128, 2, ng_dt + ng_kl], f32).ap()
    idxt = nc.alloc_sbuf_tensor("idxt", [128, S_total], i16).ap()

    # probe views: srcI reinterpreted as plane-major scratch
    srcF = srcI.rearrange("p c d -> p (c d)")
    inp_pl = inp.rearrange("(pl p) n -> p pl n", pl=2)

    # cell view of the interleaved source: cell c = tokens (2c, 2c+1)
    cellv = srcI.rearrange("p (c t) d -> p c (t d)", t=2)
    ogW4 = [w.rearrange("p (c u) -> p c u", u=4) for w in ogW]
    ogW2 = [w.rearrange("p (c u) -> p c u", u=2) for w in ogW]
    # de-interleave read views: [p, plane, token]
    deW = [w.rearrange("p (c pl) -> p pl c", pl=2) for w in ogW]

    K = n_iters
    from contextlib import ExitStack

    with ExitStack() as ctx:
        block = ctx.enter_context(nc.Block())
        sL = [ctx.enter_context(nc.semaphore(f"sL{i}")) for i in range(2)]
        sC = ctx.enter_context(nc.semaphore("sC"))     # prefix copies (+16)
        sI = ctx.enter_context(nc.semaphore("sI"))     # idx load (+16)
        sS = [ctx.enter_context(nc.semaphore(f"sS{i}")) for i in range(2)]
        vI = ctx.enter_context(nc.semaphore("vI"))     # interleaves (+1)
        vD = ctx.enter_context(nc.semaphore("vD"))     # de-interleaves (+1)
        gp = ctx.enter_context(nc.semaphore("gp"))     # gathers (+1)

        import os
        ncopy = int(os.environ.get("COPY_CHUNKS", "2"))
        cw = -(-start // ncopy)
        copies = [(c * cw, min(start, (c + 1) * cw)) for c in range(ncopy)]

        # ctest probe layout: prefix copy via SBUF bounce, reads on sync
        # ring / writes on scalar ring, ping-pong [128, 2, ccw] buffers
        # carved out of srcI scratch
        ncc = 5
        ccw = -(-start // ncc)
        cch = [(c * ccw, min(start, (c + 1) * ccw)) for c in range(ncc)]
        cbv = [srcF[:, 2 * ccw * i : 2 * ccw * (i + 1)]
               .rearrange("p (pl j) -> p pl j", pl=2) for i in range(2)]

        @block.sync
        def _(sync):
            if do_ctest:
                ln = [0, 0]
                for k in range(K):
                    for c, (lo, hi) in enumerate(cch):
                        b = c % 2
                        if ln[b] > 0:
                            # buffer reuse: its previous write drained
                            sync.wait_ge(sS[b], 16 * ln[b])
                        sync.dma_start(
                            out=cbv[b][:, :, 0 : hi - lo],
                            in_=inp_pl[:, :, lo:hi],
                        ).then_inc(sL[b], 16)
                        ln[b] += 1
                for b in range(2):
                    sync.wait_ge(sL[b], 16 * ln[b])
                return
            for k in range(K):
                if do_load:
                    for e, (lo, hi) in enumerate(subs):
                        if do_gather:
                            # slot e%2 reused from chunk e-2: its two
                            # interleaves must be done
                            w = 16 * k + 2 * (e - 1)
                            if w > 0:
                                sync.wait_ge(vI, w)
                            # self-wait (race-detector hygiene: orders the
                            # slot sem's next updates)
                            sync.wait_ge(sL[e % 2], 32 * (k * 4 + e // 2))
                        sync.dma_start(
                            out=stga[e % 2][:, 0 : hi - lo],
                            in_=inp[0:128, start + lo : start + hi],
                        ).then_inc(sL[e % 2], 16)
                        sync.dma_start(
                            out=stgb[e % 2][:, 0 : hi - lo],
                            in_=inp[128:256, start + lo : start + hi],
                        ).then_inc(sL[e % 2], 16)
                if do_copy:
                    for lo, hi in copies:
                        sync.dma_start(
                            out=out[:, lo:hi], in_=inp[:, lo:hi]
                        ).then_inc(sC, 16)
            if do_copy:
                sync.wait_ge(sC, 16 * ncopy * K)
            if do_load:
                sync.wait_ge(sL[0], 32 * 4 * K)
                sync.wait_ge(sL[1], 32 * 4 * K)

        @block.vector
        def _(vec):
            if not do_gather:
                return
            for k in range(K):
                for e, (lo, hi) in enumerate(subs):
                    # both loads of this chunk landed
                    vec.wait_ge(sL[e % 2], 32 * (k * 4 + e // 2 + 1))
                    if e == 0 and k > 0:
                        # srcI overwrite: all gathers of iter k-1 done
                        vec.wait_ge(gp, NCALL * k)
                    vec.tensor_copy(
                        srcI[:, lo:hi, 0], stga[e % 2][:, 0 : hi - lo]
                    ).then_inc(vI, 1)
                    vec.tensor_copy(
                        srcI[:, lo:hi, 1], stgb[e % 2][:, 0 : hi - lo]
                    ).then_inc(vI, 1)
                for q in range(NCALL):
                    g = NCALL * k + q
                    vec.wait_ge(gp, g + 1)           # gather (k, q) done
                    if do_store and q == 0 and k > 0:
                        # ogDe overwrite: iter k-1's store drained
                        vec.wait_ge(sS[0], 16 * k)
                    col0, n = call_tok[q]
                    vec.tensor_copy(
                        ogDe[:, :, col0 : col0 + n], deW[g % 2][:, :, 0:n]
                    ).then_inc(vD, 1)

        @block.scalar
        def _(scalar):
            if do_ctest:
                ln = [0, 0]
                for k in range(K):
                    for c, (lo, hi) in enumerate(cch):
                        b = c % 2
                        ln[b] += 1
                        scalar.wait_ge(sL[b], 16 * ln[b])
                        scalar.dma_start(
                            out=out_pl[:, :, lo:hi],
                            in_=cbv[b][:, :, 0 : hi - lo],
                        ).then_inc(sS[b], 16)
                for b in range(2):
                    scalar.wait_ge(sS[b], 16 * ln[b])
                return
            if do_stest2:
                big = srcF[:, 0 : 2 * (ng_dt + ng_kl)].rearrange(
                    "p (pl j) -> p pl j", pl=2)
                for k in range(K):
                    if k > 0:
                        scalar.wait_ge(sS[0], 16 * k)
                    scalar.dma_start(
                        out=out_pl[:, :, start : start + ng_dt + ng_kl],
                        in_=big,
                    ).then_inc(sS[0], 16)
                scalar.wait_ge(sS[0], 16 * K)
                return
            if do_gather:
                scalar.dma_start(out=idxt[:], in_=idxs[:]).then_inc(sI, 16)
            if do_store:
                ngall = ng_dt + ng_kl
                for k in range(K):
                    scalar.wait_ge(vD, NCALL * (k + 1))
                    scalar.dma_start(
                        out=out_pl[:, :, start : start + ngall],
                        in_=ogDe[:],
                    ).then_inc(sS[0], 16)
                scalar.wait_ge(sS[0], 16 * K)

        @block.gpsimd
        def _(g):
            if not do_gather:
                return
            g.load_library(library_config.ap_gather)
            g.wait_ge(sI, 16)
            for k in range(K):
                for q in range(NCALL):
                    gidx = NCALL * k + q
                    g.wait_ge(vI, 16 * k + 2 * nsub[q])
                    if gidx >= 2:
                        # window gidx%2 reuse: deint of call gidx-2 done
                        g.wait_ge(vD, gidx - 1)
                    bq = min(nelems, nsub[q] * E)
                    b = gidx % 2
                    if q < _DT_Q:
                        lo, hi = dt_qs[q]
                        bqc = min(nelems, bq + 1) // 2
                        g.ap_gather(
                            out_ap=ogW4[b][:, 0 : hi - lo, :],
                            in_ap=cellv[:, 0:bqc, :],
                            idxs_ap=idxt[:, s_off[q] : s_off[q + 1]],
                            channels=128,
                            num_elems=bqc,
                            d=4,
                            num_idxs=hi - lo,
                        ).then_inc(gp, 1)
                    else:
                        g.ap_gather(
                            out_ap=ogW2[b][:, 0:nk_pad, :],
                            in_ap=srcI[:, 0:bq, :],
                            idxs_ap=idxt[:, s_off[q] : s_off[q] + nk_pad // 16],
                            channels=128,
                            num_elems=bq,
                            d=2,
                            num_idxs=nk_pad,
                        ).then_inc(gp, 1)

    # Populate .instr bytes for extended-inst InstISA subclasses (APGather,
    # PseudoReloadLibraryIndex). Raw Bass doesn't run this pass; without it
    # walrus fails with "ISA wrong length".
    mybir.codegen_inst_isa_subclasses(nc)

    _prog_cache[key] = (nc, None)
    return nc, None


# ---------------------------------------------------------------------------
# Public entry point
# ---------------------------------------------------------------------------

def _program_and_inputs(x, mask, n_iters=1,
                        parts=("copy", "load", "gather", "store")):
    """Build (nc, in_maps) for the device path (test/bench helper)."""
    B, C, N = x.shape
    per_batch = [_batch_indices(mask[b, 0]) for b in range(B)]
    start0 = per_batch[0][0]
    rels = [r for _, r, _ in per_batch]
    splits = [_split_dt_kl(r) for r in rels]
    nd0, nk0 = len(splits[0][0]), len(splits[0][1])
    M = max(s + len(r) for s, r, _ in per_batch)
    nsub = _source_bounds(rels, N - start0)
    nc, _ = _build_program(C, N, start0, nd0, nk0, M, n_iters, nsub,
                           parts=parts)
    in_maps = [
        {
            "input": np.ascontiguousarray(x[b]),
            "idxs": _make_idx_input(rels[b]),
        }
        for b in range(B)
    ]
    return nc, in_maps


def kernel(input, pooling_mask, _n_iters=1):
    x = np.asarray(input)
    mask = np.asarray(pooling_mask)
    B, C, N = x.shape
    assert x.dtype == np.float32

    per_batch = [_batch_indices(mask[b, 0]) for b in range(B)]
    M = max(s + len(r) for s, r, _ in per_batch)
    device_ok = (
        B == _NUM_CORES
        and C == 256
        and len({s for s, _, _ in per_batch}) == 1
        and len({len(r) for _, r, _ in per_batch}) == 1
    )
    start0 = per_batch[0][0]
    if device_ok:
        splits = [_split_dt_kl(r) for _, r, _ in per_batch]
        nd0, nk0 = len(splits[0][0]), len(splits[0][1])
        device_ok = (
            0 < (N - start0) * 2 <= 2 ** 15
            and nd0 > 8 * 32 * 2        # 4 quarters of >=32 cells
            and nk0 > 0
            and all(len(dt) == nd0 and len(kl) == nk0 for dt, kl in splits)
            # dt part must be even-aligned contiguous pairs
            and all(
                len(dt) % 2 == 0
                and (dt[0::2] % 2 == 0).all()
                and (dt[1::2] == dt[0::2] + 1).all()
                for dt, _ in splits
            )
        )
    if not device_ok:
        # Irregular structure (not produced by this module's mask builder)
        # - fall back to a host gather.
        out = np.zeros((B, C, M), np.float32)
        for b, (s, rel, _) in enumerate(per_batch):
            n = s + len(rel)
            g = np.concatenate([np.arange(s, dtype=np.int64), s + rel])
            out[b, :, :n] = x[b][:, g]
        return out

    rels = [r for _, r, _ in per_batch]
    nsub = _source_bounds(rels, N - start0)
    nc, _ = _build_program(C, N, start0, nd0, nk0, M, _n_iters, nsub)
    in_maps = [
        {
            "input": np.ascontiguousarray(x[b]),
            "idxs": _make_idx_input(rels[b]),
        }
        for b in range(B)
    ]
    res = run_bass_kernel_spmd(nc, in_maps, list(range(_NUM_CORES)))
    return np.stack([res.results[b]["output"] for b in range(B)])
